# revision 1
# baseline (speedup 1.0000x reference)
"""ConcatAttentionUnit (Bahdanau additive attention) Trainium2 Bass kernel.

Math (per batch b):
    sq = hq @ W1                  [Q=512, V=256]
    sp = hp @ W2                  [P=512, V=256]
    s[p,q]  = sum_v v[v] * tanh(sp[p,v] + sq[q,v])
    a = softmax_q(s); out = a @ hq            [P, 512]

Sharding: data-parallel over (batch, p-half): 8 cores, each handles one
(b, 256-row slice of p). No collectives; full inputs in, full output out.

Per-core kernel (roofline engine = ScalarE: B*P*Q*V/8 = 33.5M tanh
elements at 1 elem/lane/cycle @1.2GHz = 218us floor; measured ~222us
back-to-back, ~261us total):
  - hq/hp transposed via PE(+identity), projected to sqT[v,q] (bf16) and
    spT[v,p] (f32) with v on partitions.
  - broadcast add: DVE tensor_scalar_add(in0=sqT chunk [128v,512q],
    scalar=spT[:,p] per-partition column) -> bf16 tanh-input buffer;
    FD=512 amortizes the ~128cyc per-instruction scalar register load.
  - tanh: one ACT instruction per group of G=16 p rows (FD=16384)
    amortizes ScalarE fixed overhead; in-place on the buffer.
  - v-dot: M=128/N=1 bf16 matmuls, lhsT = tanh tile [128v,128q],
    rhs = v chunk [128,1], accumulated into PSUM *columns* p of four
    [128q, 256p] banks -> s^T[q,p] (PSUM matmul base-partition must be
    0/32/64, so s is built transposed; exp(s^T) is then directly the
    lhsT of the output matmul - no transpose of the attention matrix).
  - softmax: exp on ACT; partition sums Z[p] via ones-matmuls; the
    softmax normalization is folded into the epilogue as a per-partition
    multiply by 1/Z after  o = exp(s)^T-matmul-hq  (all bf16 matmuls).
Emission order == Tile scheduler priority: phase A is emitted in strict
dependency order (later-needed DMAs after the transposes that gate the
first tanh), head/tail GROUPS are tapered so the first tanh starts early
and the last group exposes few trailing matmuls.
"""

import ml_dtypes
import numpy as np

import concourse.bass as bass  # noqa: F401  (registers rust bindings)
import concourse.mybir as mybir
import concourse.tile as tile
from concourse import bacc
from concourse.bass_utils import run_bass_kernel_spmd
F32 = mybir.dt.float32
BF16 = mybir.dt.bfloat16
AF = mybir.ActivationFunctionType

B, Q, P, D, E, V = 4, 512, 512, 512, 512, 256
NCORES = 8
PSH = P * B // NCORES  # 256 p rows per core
QC = Q // 128  # q chunks
DC = D // 128  # d chunks
EC = E // 128  # e chunks
VC = V // 128  # v chunks
PC = PSH // 128  # p chunks
# p rows per ACT tanh instruction group (FD = g*VC*Q; 16 -> 16384). Small
# leading groups let the first tanh start before a full group of pre-adds.
GROUPS = [4, 4, 8] + [16] * 14 + [8, 4, 4]
assert sum(GROUPS) == PSH


def kernel_body(nc, tc, hq, hp, w1, w2, vv, identity, out):
    with (
        tc.tile_pool(name="persist", bufs=1) as pp,
        tc.tile_pool(name="tmp", bufs=1) as tp,
        tc.tile_pool(name="tanhbuf", bufs=3) as bp,
        tc.tile_pool(name="fin", bufs=2) as fin,
    ):
        # transposes + projections use 2 scratch PSUM banks, released before
        # the epilogue pools are opened (4 sT + 2 z + 2 o = 8 banks total).
        ps_w = tc.alloc_tile_pool(name="ps_w", bufs=4, space="PSUM")
        # ---- phase A: loads, casts, transposes, projections ----
        # Emission order == scheduler priority order: anything emitted after
        # a DMA inherits semaphore waits behind it, so emit strictly in
        # dependency order (ident/hq/hp loads -> casts -> PE transposes ->
        # w/v loads -> projections).
        ident = pp.tile([128, 128], BF16, tag="ident")
        nc.sync.dma_start(ident[:], identity[:, :])
        hq_f32 = tp.tile([128, QC * D], F32, tag="hq_f32")
        for qc in range(QC):
            nc.sync.dma_start(
                hq_f32[:, qc * D : (qc + 1) * D], hq[qc * 128 : (qc + 1) * 128, :]
            )
        hp_f32 = tp.tile([128, PC * E], F32, tag="hp_f32")
        for pc in range(PC):
            nc.sync.dma_start(
                hp_f32[:, pc * E : (pc + 1) * E], hp[pc * 128 : (pc + 1) * 128, :]
            )
        w1_f32 = tp.tile([128, DC * V], F32, tag="w1_f32")
        for dc in range(DC):
            nc.scalar.dma_start(
                w1_f32[:, dc * V : (dc + 1) * V], w1[dc * 128 : (dc + 1) * 128, :]
            )
        w2_f32 = tp.tile([128, EC * V], F32, tag="w2_f32")
        for ec in range(EC):
            nc.scalar.dma_start(
                w2_f32[:, ec * V : (ec + 1) * V], w2[ec * 128 : (ec + 1) * 128, :]
            )
        hq_bf = pp.tile([128, QC * D], BF16, tag="hq_bf")
        for qc in range(QC):
            nc.vector.tensor_copy(
                hq_bf[:, qc * D : (qc + 1) * D], hq_f32[:, qc * D : (qc + 1) * D]
            )
        hp_bf = tp.tile([128, PC * E], BF16, tag="hp_bf")
        for pc in range(PC):
            nc.vector.tensor_copy(
                hp_bf[:, pc * E : (pc + 1) * E], hp_f32[:, pc * E : (pc + 1) * E]
            )

        hqT = tp.tile([128, DC * Q], BF16, tag="hqT")
        for qc in range(QC):
            for dc in range(DC):
                ps = ps_w.tile([128, 128], BF16, tag="work")
                nc.tensor.transpose(
                    ps[:],
                    hq_bf[:, qc * D + dc * 128 : qc * D + (dc + 1) * 128],
                    ident[:],
                )
                nc.vector.tensor_copy(
                    hqT[:, dc * Q + qc * 128 : dc * Q + (qc + 1) * 128], ps[:]
                )
        hpT = tp.tile([128, EC * PSH], BF16, tag="hpT")
        for pc in range(PC):
            for ec in range(EC):
                ps = ps_w.tile([128, 128], BF16, tag="work")
                nc.tensor.transpose(
                    ps[:],
                    hp_bf[:, pc * E + ec * 128 : pc * E + (ec + 1) * 128],
                    ident[:],
                )
                nc.scalar.copy(
                    hpT[:, ec * PSH + pc * 128 : ec * PSH + (pc + 1) * 128], ps[:]
                )

        w1_bf = tp.tile([128, DC * V], BF16, tag="w1_bf")
        nc.scalar.copy(w1_bf[:], w1_f32[:])
        w2_bf = tp.tile([128, EC * V], BF16, tag="w2_bf")
        nc.scalar.copy(w2_bf[:], w2_f32[:])
        v_f32 = tp.tile([128, VC], F32, tag="v_f32")
        for c in range(VC):
            nc.sync.dma_start(v_f32[:, c : c + 1], vv[c * 128 : (c + 1) * 128, :])
        v_bf = pp.tile([128, VC], BF16, tag="v_bf")
        nc.scalar.copy(v_bf[:], v_f32[:])

        # projections: sqT[v,q] (bf16, streamed operand of the pre-add) and
        # spT[v,p] (f32, the per-p tensor_scalar operand source)
        sqT = pp.tile([128, VC * Q], BF16, tag="sqT")
        sq_ps = [
            ps_w.tile([128, 512], F32, tag=f"prj{vc}", name=f"sq_ps{vc}", bufs=1)
            for vc in range(VC)
        ]
        for dc in range(DC):
            for vc in range(VC):
                nc.tensor.matmul(
                    sq_ps[vc][:, :Q],
                    w1_bf[:, dc * V + vc * 128 : dc * V + (vc + 1) * 128],
                    hqT[:, dc * Q : (dc + 1) * Q],
                    start=(dc == 0),
                    stop=(dc == DC - 1),
                )
        for vc in range(VC):
            nc.vector.tensor_copy(sqT[:, vc * Q : (vc + 1) * Q], sq_ps[vc][:, :Q])
        spT = pp.tile([128, VC * PSH], F32, tag="spT")
        sp_ps = [
            ps_w.tile([128, 512], F32, tag=f"prj2{vc}", name=f"sp_ps{vc}", bufs=1)
            for vc in range(VC)
        ]
        for ec in range(EC):
            for vc in range(VC):
                nc.tensor.matmul(
                    sp_ps[vc][:, :PSH],
                    w2_bf[:, ec * V + vc * 128 : ec * V + (vc + 1) * 128],
                    hpT[:, ec * PSH : (ec + 1) * PSH],
                    start=(ec == 0),
                    stop=(ec == EC - 1),
                )
        for vc in range(VC):
            nc.scalar.copy(spT[:, vc * PSH : (vc + 1) * PSH], sp_ps[vc][:, :PSH])
        ones_bf = pp.tile([128, 1], BF16, tag="ones_bf")
        nc.vector.memset(ones_bf[:], 1.0)
        ps_w.release()
        ps_s_pool = tc.alloc_tile_pool(name="ps_s", bufs=1, space="PSUM")
        ps_o_pool = tc.alloc_tile_pool(name="ps_o", bufs=1, space="PSUM")
        ps_z_pool = tc.alloc_tile_pool(name="ps_z", bufs=1, space="PSUM")

        # ---------------- main loop: tanh scores, sT[q, p] in PSUM ----------
        # buf[v, q] = tanh(sqT[v, q] + spT[v, p]) for each p; the per-p value
        # rides the tensor_scalar per-partition operand (FD=512 amortizes the
        # ~128cyc scalar register load), tanh batched over G p's per ACT instr.
        sT_ps = [
            ps_s_pool.tile([128, PSH], F32, tag=f"sT{qc}", name=f"sT_ps{qc}")
            for qc in range(QC)
        ]
        p0 = 0
        for gsz in GROUPS:
            buf = bp.tile([128, gsz * VC * Q], BF16, tag="buf")
            for g in range(gsz):
                p = p0 + g
                for vc in range(VC):
                    nc.vector.tensor_scalar_add(
                        buf[:, (g * VC + vc) * Q : (g * VC + vc + 1) * Q],
                        sqT[:, vc * Q : (vc + 1) * Q],
                        spT[:, vc * PSH + p : vc * PSH + p + 1],
                    )
            nc.scalar.activation(buf[:], buf[:], AF.Tanh)
            for g in range(gsz):
                p = p0 + g
                for vc in range(VC):
                    for qc in range(QC):
                        off = (g * VC + vc) * Q + qc * 128
                        nc.tensor.matmul(
                            sT_ps[qc][:, p : p + 1],
                            buf[:, off : off + 128],
                            v_bf[:, vc : vc + 1],
                            start=(vc == 0),
                            stop=(vc == VC - 1),
                        )
            p0 += gsz

        # ---------------- softmax (unnormalized) + output ----------------
        # exp(sT)[q, p] is directly the lhsT of the final matmul; Z[p] via
        # ones-matmuls (partition-dim sum); normalization folded into epilogue.
        exp_sT = pp.tile([128, QC * PSH], BF16, tag="exp_sT")
        rec = pp.tile([128, PC], F32, tag="rec")
        for qc in range(QC):
            nc.scalar.activation(
                exp_sT[:, qc * PSH : (qc + 1) * PSH], sT_ps[qc][:], AF.Exp
            )
        for pc in range(PC):
            z_ps = ps_z_pool.tile([128, 1], F32, tag=f"z{pc}", name=f"z_ps{pc}")
            for qc in range(QC):
                nc.tensor.matmul(
                    z_ps[:],
                    exp_sT[:, qc * PSH + pc * 128 : qc * PSH + (pc + 1) * 128],
                    ones_bf[:],
                    start=(qc == 0),
                    stop=(qc == QC - 1),
                )
            nc.vector.reciprocal(rec[:, pc : pc + 1], z_ps[:])
            o_ps = ps_o_pool.tile([128, D], F32, tag=f"o{pc}", name=f"o_ps{pc}")
            for qc in range(QC):
                nc.tensor.matmul(
                    o_ps[:],
                    exp_sT[:, qc * PSH + pc * 128 : qc * PSH + (pc + 1) * 128],
                    hq_bf[:, qc * D : (qc + 1) * D],
                    start=(qc == 0),
                    stop=(qc == QC - 1),
                )
            ob = fin.tile([128, D], F32, tag="ob")
            nc.vector.tensor_scalar_mul(ob[:], o_ps[:], rec[:, pc : pc + 1])
            nc.sync.dma_start(out[pc * 128 : (pc + 1) * 128, :], ob[:])
        ps_z_pool.release()
        ps_o_pool.release()
        ps_s_pool.release()


def build_program():
    nc = bacc.Bacc("TRN2", target_bir_lowering=False, debug=False)
    hq = nc.dram_tensor("hq_b", [Q, D], F32, kind="ExternalInput")
    hp = nc.dram_tensor("hp_s", [PSH, E], F32, kind="ExternalInput")
    w1 = nc.dram_tensor("W1", [D, V], F32, kind="ExternalInput")
    w2 = nc.dram_tensor("W2", [E, V], F32, kind="ExternalInput")
    vv = nc.dram_tensor("v", [V, 1], F32, kind="ExternalInput")
    identity = nc.dram_tensor("identity", [128, 128], BF16, kind="ExternalInput")
    out = nc.dram_tensor("out", [PSH, D], F32, kind="ExternalOutput")
    with tile.TileContext(nc) as tc:
        kernel_body(nc, tc, hq, hp, w1, w2, vv, identity, out)
    nc.compile()
    return nc


_PROGRAM = None


def _get_program():
    global _PROGRAM
    if _PROGRAM is None:
        _PROGRAM = build_program()
    return _PROGRAM


def make_in_maps(hq, hp, W1, W2, v):
    w1 = np.ascontiguousarray(W1, dtype=np.float32)
    w2 = np.ascontiguousarray(W2, dtype=np.float32)
    vv = np.ascontiguousarray(v, dtype=np.float32).reshape(V, 1)
    ident = np.eye(128, dtype=ml_dtypes.bfloat16)
    in_maps = []
    for c in range(NCORES):
        b = c // (NCORES // B)
        ph = c % (NCORES // B)
        in_maps.append(
            {
                "hq_b": np.ascontiguousarray(hq[b], dtype=np.float32),
                "hp_s": np.ascontiguousarray(
                    hp[b, ph * PSH : (ph + 1) * PSH], dtype=np.float32
                ),
                "W1": w1,
                "W2": w2,
                "v": vv,
                "identity": ident,
            }
        )
    return in_maps


def kernel(hq, hp, W1, W2, v, _trace=False, _return_raw=False, _tmpdir=None):
    nc = _get_program()
    in_maps = make_in_maps(hq, hp, W1, W2, v)
    res = run_bass_kernel_spmd(
        nc, in_maps, list(range(NCORES)), trace=_trace, tmpdir=_tmpdir
    )
    out = np.empty((B, P, D), dtype=np.float32)
    for c in range(NCORES):
        b = c // (NCORES // B)
        ph = c % (NCORES // B)
        out[b, ph * PSH : (ph + 1) * PSH, :] = res.results[c]["out"]
    if _return_raw:
        return out, res
    return out



# revision 8
# speedup vs baseline: 2.8012x; 2.8012x over previous
"""ConcatAttentionUnit (Bahdanau additive attention) Trainium2 Bass kernel.

Math (per batch b):
    sq = hq @ W1                  [Q=512, V=256]
    sp = hp @ W2                  [P=512, V=256]
    s[p,q]  = sum_v v[v] * tanh(sp[p,v] + sq[q,v])
    a = softmax_q(s); out = a @ hq            [P, 512]

Key trick: tanh(x+y) ~= sum_k c_k sin(w_k (x+y)) with w_k = k*pi/L
(weighted LSQ fit, L=9, K=10).  The angle-addition identity
    sin(w(x+y)) = sin(wx + pi/4) sin(wy + pi/4)
               -  sin(wx - pi/4) sin(wy - pi/4)
factorizes the P*Q*V tanh volume into a rank 2*K*V PE matmul.  This
replaces the baseline's per-core 33.5M-element ScalarE tanh roofline
(~218us) with ~41k PE cycles + ~31k ACT cycles + ~31k DVE cycles.

HW ACT Sin has no range reduction (garbage beyond ~+-pi), so each
harmonic's phase is reduced once per side on DVE via the f32
magic-number round (M = 1.5*2^23):
    t = x * (w_k/2pi);  r = (t + M) - M  (= round t);  w = t - r
then ACT evaluates sin(2pi*w +- pi/4) with args in [-1.25pi, 1.25pi]
(mode "p4pair"), or sin/cos via bias pi/2 (mode "cosbias"), or a second
reduction shifted by 0.25 (mode "dualred", args strictly within +-pi).

Sharding: data-parallel over (batch, p-half): 8 cores, each handles one
(b, 256-row slice of p). No collectives; full inputs in, full output out.

Per-core phases:
  A: load/cast/PE-transpose hq,hp; project to sqT[v,q], spT[v,p] (f32).
  B: per k: DVE phase reduction; ACT sin -> bf16 basis; per-partition
     weighting by +-c_k*v_v (ACT Identity with AP scale).
  C: 2*K*VC*QC matmuls accumulate s^T[q,p] into 4 PSUM banks.
  D: exp on ACT (exp(s^T) is directly the lhsT of the output matmul);
     Z[p] via ones-matmuls; out = exp(s)^T @ hq; normalize by 1/Z
     (per-partition multiply) in the epilogue; DMA out.
"""

import ml_dtypes
import numpy as np

import concourse.bass as bass  # noqa: F401  (registers rust bindings)
import concourse.mybir as mybir
import concourse.tile as tile
from concourse import bacc
from concourse.bass_utils import run_bass_kernel_spmd

F32 = mybir.dt.float32
BF16 = mybir.dt.bfloat16
AF = mybir.ActivationFunctionType
ALU = mybir.AluOpType

B, Q, P, D, E, V = 4, 512, 512, 512, 512, 256
NCORES = 8
PSH = P * B // NCORES  # 256 p rows per core
QC = Q // 128
DC = D // 128
EC = E // 128
VC = V // 128
PC = PSH // 128

K_TERMS = 10
L_PERIOD = 9.0
# p4pair basis with s1 negation via scale sign (see main loop comment)
MAGIC = 1.5 * 2.0**23


def _fit_coeffs(K=K_TERMS, L=L_PERIOD, sigma=np.sqrt(2.0), floor=1e-4):
    x = np.linspace(0, L, 4001)
    w = np.exp(-(x**2) / (2 * sigma**2)) + floor
    A = np.sin(np.outer(x, np.arange(1, K + 1) * np.pi / L))
    t = np.tanh(x)
    sw = np.sqrt(w)[:, None]
    c, *_ = np.linalg.lstsq(A * sw, t * sw[:, 0], rcond=None)
    return c  # [K]


COEFFS = _fit_coeffs()
OMEGAS = np.arange(1, K_TERMS + 1) * np.pi / L_PERIOD

QW = Q * VC  # 1024 cols per phase on the q side
PW = PSH * VC  # 512 cols per phase on the p side


def kernel_body(nc, tc, hq, hp, w1, w2, vv, identity, out):
    with (
        tc.tile_pool(name="persist", bufs=1) as pp,
        tc.tile_pool(name="tmp", bufs=1) as tp,
        tc.tile_pool(name="red", bufs=3) as rp,
        tc.tile_pool(name="bas", bufs=3) as bp,
        tc.tile_pool(name="fin", bufs=2) as fin,
    ):
        ps_w = tc.alloc_tile_pool(name="ps_w", bufs=4, space="PSUM")
        # ---- phase A: loads, casts, transposes, projections ----
        ident = pp.tile([128, 128], BF16, tag="ident")
        nc.sync.dma_start(ident[:], identity[:, :])
        hq_f32 = tp.tile([128, QC * D], F32, tag="hq_f32")
        for qc in range(QC):
            nc.sync.dma_start(
                hq_f32[:, qc * D : (qc + 1) * D], hq[qc * 128 : (qc + 1) * 128, :]
            )
        hp_f32 = tp.tile([128, PC * E], F32, tag="hp_f32")
        for pc in range(PC):
            nc.sync.dma_start(
                hp_f32[:, pc * E : (pc + 1) * E], hp[pc * 128 : (pc + 1) * 128, :]
            )
        w1_f32 = tp.tile([128, DC * V], F32, tag="w1_f32")
        for dc in range(DC):
            nc.scalar.dma_start(
                w1_f32[:, dc * V : (dc + 1) * V], w1[dc * 128 : (dc + 1) * 128, :]
            )
        w2_f32 = tp.tile([128, EC * V], F32, tag="w2_f32")
        for ec in range(EC):
            nc.scalar.dma_start(
                w2_f32[:, ec * V : (ec + 1) * V], w2[ec * 128 : (ec + 1) * 128, :]
            )
        hq_bf = pp.tile([128, QC * D], BF16, tag="hq_bf")
        for qc in range(QC):
            nc.vector.tensor_copy(
                hq_bf[:, qc * D : (qc + 1) * D], hq_f32[:, qc * D : (qc + 1) * D]
            )
        hp_bf = tp.tile([128, PC * E], BF16, tag="hp_bf")
        for pc in range(PC):
            nc.vector.tensor_copy(
                hp_bf[:, pc * E : (pc + 1) * E], hp_f32[:, pc * E : (pc + 1) * E]
            )

        hqT = tp.tile([128, DC * Q], BF16, tag="hqT")
        for qc in range(QC):
            for dc in range(DC):
                ps = ps_w.tile([128, 128], BF16, tag="work")
                nc.tensor.transpose(
                    ps[:],
                    hq_bf[:, qc * D + dc * 128 : qc * D + (dc + 1) * 128],
                    ident[:],
                )
                nc.vector.tensor_copy(
                    hqT[:, dc * Q + qc * 128 : dc * Q + (qc + 1) * 128], ps[:]
                )
        hpT = tp.tile([128, EC * PSH], BF16, tag="hpT")
        for pc in range(PC):
            for ec in range(EC):
                ps = ps_w.tile([128, 128], BF16, tag="work")
                nc.tensor.transpose(
                    ps[:],
                    hp_bf[:, pc * E + ec * 128 : pc * E + (ec + 1) * 128],
                    ident[:],
                )
                nc.scalar.copy(
                    hpT[:, ec * PSH + pc * 128 : ec * PSH + (pc + 1) * 128], ps[:]
                )

        w1_bf = tp.tile([128, DC * V], BF16, tag="w1_bf")
        nc.scalar.copy(w1_bf[:], w1_f32[:])
        w2_bf = tp.tile([128, EC * V], BF16, tag="w2_bf")
        nc.scalar.copy(w2_bf[:], w2_f32[:])
        v_f32 = tp.tile([128, VC], F32, tag="v_f32")
        for c in range(VC):
            nc.sync.dma_start(v_f32[:, c : c + 1], vv[c * 128 : (c + 1) * 128, :])
        # weight table: vck[:, (vc, k)] = c_k * v[vc-chunk]
        vck = pp.tile([128, VC * K_TERMS], F32, tag="vck")
        for c in range(VC):
            for k in range(K_TERMS):
                col = c * K_TERMS + k
                nc.vector.tensor_scalar_mul(
                    vck[:, col : col + 1],
                    v_f32[:, c : c + 1],
                    float(COEFFS[k]),
                )

        # projections: sqT[v,q] and spT[v,p], both f32 (basis inputs)
        sqT = pp.tile([128, VC * Q], F32, tag="sqT")
        sq_ps = [
            ps_w.tile([128, 512], F32, tag=f"prj{vc}", name=f"sq_ps{vc}", bufs=1)
            for vc in range(VC)
        ]
        for dc in range(DC):
            for vc in range(VC):
                nc.tensor.matmul(
                    sq_ps[vc][:, :Q],
                    w1_bf[:, dc * V + vc * 128 : dc * V + (vc + 1) * 128],
                    hqT[:, dc * Q : (dc + 1) * Q],
                    start=(dc == 0),
                    stop=(dc == DC - 1),
                )
        for vc in range(VC):
            nc.vector.tensor_copy(sqT[:, vc * Q : (vc + 1) * Q], sq_ps[vc][:, :Q])
        spT = pp.tile([128, VC * PSH], F32, tag="spT")
        sp_ps = [
            ps_w.tile([128, 512], F32, tag=f"prj2{vc}", name=f"sp_ps{vc}", bufs=1)
            for vc in range(VC)
        ]
        for ec in range(EC):
            for vc in range(VC):
                nc.tensor.matmul(
                    sp_ps[vc][:, :PSH],
                    w2_bf[:, ec * V + vc * 128 : ec * V + (vc + 1) * 128],
                    hpT[:, ec * PSH : (ec + 1) * PSH],
                    start=(ec == 0),
                    stop=(ec == EC - 1),
                )
        for vc in range(VC):
            nc.scalar.copy(spT[:, vc * PSH : (vc + 1) * PSH], sp_ps[vc][:, :PSH])
        ones_bf = pp.tile([128, 1], BF16, tag="ones_bf")
        nc.vector.memset(ones_bf[:], 1.0)
        bias_p4 = pp.tile([128, 1], F32, tag="bias_p4")
        nc.vector.memset(bias_p4[:], np.pi / 4)
        bias_m4 = pp.tile([128, 1], F32, tag="bias_m4")
        nc.vector.memset(bias_m4[:], -np.pi / 4)
        ps_w.release()
        ps_s_pool = tc.alloc_tile_pool(name="ps_s", bufs=1, space="PSUM")
        ps_o_pool = tc.alloc_tile_pool(name="ps_o", bufs=1, space="PSUM")
        ps_z_pool = tc.alloc_tile_pool(name="ps_z", bufs=1, space="PSUM")

        # ---------------- main loop: trig basis + rank-2KV matmul ----------
        # Per k: reduce phase once per side (wn = round(t) - t = -w, via one
        # dual-op tensor_scalar reading the projection + one stt reading the
        # ACT-produced t); then 4 Sin evals:
        #   qb_s0 = Sin(wn_q, -2pi, +pi/4) =  sin(2pi w_q + pi/4)  (= Q+)
        #   qb_s1 = Sin(wn_q, +2pi, +pi/4) = -sin(2pi w_q - pi/4)  (= -Q-)
        #   pb_s0 = Sin(wn_p, -2pi, +pi/4) =  P+
        #   pb_s1 = Sin(wn_p, -2pi, -pi/4) =  P-
        # s = sum_k c_k [Q+ P+ - Q- P-]; all weights are +c_k*v_v, applied on
        # the p side with one [128,512] per-partition multiply per vc (pbw is
        # laid out vc-major so s0|s1 share the scalar).
        sT_ps = [
            ps_s_pool.tile([128, PSH], F32, tag=f"sT{qc}", name=f"sT_ps{qc}")
            for qc in range(QC)
        ]
        for k in range(K_TERMS):
            ck = float(OMEGAS[k] / (2 * np.pi))
            # ---- phase reduction ----
            a_q = rp.tile([128, QW], F32, tag="a_q")
            nc.vector.tensor_scalar(a_q[:], sqT[:], ck, MAGIC, ALU.mult, ALU.add)
            t_q = rp.tile([128, QW], F32, tag="t_q")
            nc.scalar.mul(t_q[:], sqT[:], ck)
            wn_q = rp.tile([128, QW], F32, tag="wn_q")
            nc.vector.scalar_tensor_tensor(
                wn_q[:], a_q[:], MAGIC, t_q[:], ALU.subtract, ALU.subtract
            )
            a_p = rp.tile([128, PW], F32, tag="a_p")
            nc.vector.tensor_scalar(a_p[:], spT[:], ck, MAGIC, ALU.mult, ALU.add)
            t_p = rp.tile([128, PW], F32, tag="t_p")
            nc.scalar.mul(t_p[:], spT[:], ck)
            wn_p = rp.tile([128, PW], F32, tag="wn_p")
            nc.vector.scalar_tensor_tensor(
                wn_p[:], a_p[:], MAGIC, t_p[:], ALU.subtract, ALU.subtract
            )

            # ---- ACT basis (bf16 out): p side per (s, vc) into vc-major ----
            pb = bp.tile([128, 2 * PW], BF16, tag="pb")
            qb = bp.tile([128, 2 * QW], BF16, tag="qb")
            pbw = bp.tile([128, 2 * PW], BF16, tag="pbw")
            for s, bias_t in ((0, bias_p4), (1, bias_m4)):
                for c in range(VC):
                    nc.scalar.activation(
                        pb[:, (c * 2 + s) * PSH : (c * 2 + s + 1) * PSH],
                        wn_p[:, c * PSH : (c + 1) * PSH],
                        AF.Sin, scale=-2 * np.pi, bias=bias_t[:, 0:1],
                    )
            for c in range(VC):
                nc.vector.tensor_scalar_mul(
                    pbw[:, c * 2 * PSH : (c + 1) * 2 * PSH],
                    pb[:, c * 2 * PSH : (c + 1) * 2 * PSH],
                    vck[:, c * K_TERMS + k : c * K_TERMS + k + 1],
                )
            for s, scl in ((0, -2 * np.pi), (1, 2 * np.pi)):
                nc.scalar.activation(
                    qb[:, s * QW : (s + 1) * QW], wn_q[:], AF.Sin,
                    scale=scl, bias=bias_p4[:, 0:1],
                )
                # ---- matmuls for this (k, s) ----
                for c in range(VC):
                    for qc in range(QC):
                        nc.tensor.matmul(
                            sT_ps[qc][:, :PSH],
                            qb[:, s * QW + c * Q + qc * 128 : s * QW + c * Q + (qc + 1) * 128],
                            pbw[:, (c * 2 + s) * PSH : (c * 2 + s + 1) * PSH],
                            start=(k == 0 and s == 0 and c == 0),
                            stop=(k == K_TERMS - 1 and s == 1 and c == VC - 1),
                        )

        # ---------------- softmax (unnormalized) + output ----------------
        exp_sT = pp.tile([128, QC * PSH], BF16, tag="exp_sT")
        rec = pp.tile([128, PC], F32, tag="rec")
        for qc in range(QC):
            nc.scalar.activation(
                exp_sT[:, qc * PSH : (qc + 1) * PSH], sT_ps[qc][:], AF.Exp
            )
        for pc in range(PC):
            z_ps = ps_z_pool.tile([128, 1], F32, tag=f"z{pc}", name=f"z_ps{pc}")
            for qc in range(QC):
                nc.tensor.matmul(
                    z_ps[:],
                    exp_sT[:, qc * PSH + pc * 128 : qc * PSH + (pc + 1) * 128],
                    ones_bf[:],
                    start=(qc == 0),
                    stop=(qc == QC - 1),
                )
            nc.vector.reciprocal(rec[:, pc : pc + 1], z_ps[:])
            o_ps = ps_o_pool.tile([128, D], F32, tag=f"o{pc}", name=f"o_ps{pc}")
            for qc in range(QC):
                nc.tensor.matmul(
                    o_ps[:],
                    exp_sT[:, qc * PSH + pc * 128 : qc * PSH + (pc + 1) * 128],
                    hq_bf[:, qc * D : (qc + 1) * D],
                    start=(qc == 0),
                    stop=(qc == QC - 1),
                )
            ob = fin.tile([128, D], F32, tag="ob")
            nc.vector.tensor_scalar_mul(ob[:], o_ps[:], rec[:, pc : pc + 1])
            nc.sync.dma_start(out[pc * 128 : (pc + 1) * 128, :], ob[:])
        ps_z_pool.release()
        ps_o_pool.release()
        ps_s_pool.release()


def build_program():
    nc = bacc.Bacc("TRN2", target_bir_lowering=False, debug=False)
    hq = nc.dram_tensor("hq_b", [Q, D], F32, kind="ExternalInput")
    hp = nc.dram_tensor("hp_s", [PSH, E], F32, kind="ExternalInput")
    w1 = nc.dram_tensor("W1", [D, V], F32, kind="ExternalInput")
    w2 = nc.dram_tensor("W2", [E, V], F32, kind="ExternalInput")
    vv = nc.dram_tensor("v", [V, 1], F32, kind="ExternalInput")
    identity = nc.dram_tensor("identity", [128, 128], BF16, kind="ExternalInput")
    out = nc.dram_tensor("out", [PSH, D], F32, kind="ExternalOutput")
    with tile.TileContext(nc) as tc:
        kernel_body(nc, tc, hq, hp, w1, w2, vv, identity, out)
    nc.compile()
    return nc


_PROGRAM = None


def _get_program():
    global _PROGRAM
    if _PROGRAM is None:
        _PROGRAM = build_program()
    return _PROGRAM


def make_in_maps(hq, hp, W1, W2, v):
    w1 = np.ascontiguousarray(W1, dtype=np.float32)
    w2 = np.ascontiguousarray(W2, dtype=np.float32)
    vv = np.ascontiguousarray(v, dtype=np.float32).reshape(V, 1)
    ident = np.eye(128, dtype=ml_dtypes.bfloat16)
    in_maps = []
    for c in range(NCORES):
        b = c // (NCORES // B)
        ph = c % (NCORES // B)
        in_maps.append(
            {
                "hq_b": np.ascontiguousarray(hq[b], dtype=np.float32),
                "hp_s": np.ascontiguousarray(
                    hp[b, ph * PSH : (ph + 1) * PSH], dtype=np.float32
                ),
                "W1": w1,
                "W2": w2,
                "v": vv,
                "identity": ident,
            }
        )
    return in_maps


def kernel(hq, hp, W1, W2, v, _trace=False, _return_raw=False, _tmpdir=None):
    nc = _get_program()
    in_maps = make_in_maps(hq, hp, W1, W2, v)
    res = run_bass_kernel_spmd(
        nc, in_maps, list(range(NCORES)), trace=_trace, tmpdir=_tmpdir
    )
    out = np.empty((B, P, D), dtype=np.float32)
    for c in range(NCORES):
        b = c // (NCORES // B)
        ph = c % (NCORES // B)
        out[b, ph * PSH : (ph + 1) * PSH, :] = res.results[c]["out"]
    if _return_raw:
        return out, res
    return out


def _selfcheck():
    """Numpy mimic of the exact on-device emission, vs the Fourier sum."""
    rng = np.random.default_rng(0)
    x = rng.normal(size=4000) * 1.45
    y = rng.normal(size=4000) * 1.45
    ref = np.zeros_like(x)
    for k in range(K_TERMS):
        ref += COEFFS[k] * np.sin(OMEGAS[k] * (x + y))
    acc = np.zeros_like(x)
    for k in range(K_TERMS):
        ck = OMEGAS[k] / (2 * np.pi)
        wn_q = np.round(x * ck) - x * ck
        wn_p = np.round(y * ck) - y * ck
        qb0 = np.sin(wn_q * (-2 * np.pi) + np.pi / 4)
        qb1 = np.sin(wn_q * (2 * np.pi) + np.pi / 4)
        pb0 = np.sin(wn_p * (-2 * np.pi) + np.pi / 4)
        pb1 = np.sin(wn_p * (-2 * np.pi) - np.pi / 4)
        acc += COEFFS[k] * (qb0 * pb0 + qb1 * pb1)
    print(f"  emission vs Fourier sum: {np.abs(acc - ref).max():.3e}")
    print(f"  Fourier sum vs tanh:     {np.abs(ref - np.tanh(x + y)).max():.3e}")


if __name__ == "__main__":
    _selfcheck()
